# revision 37
# baseline (speedup 1.0000x reference)
"""HGNN conv kernel for Trainium2, data-parallel over time across 8 cores.

Per core (t = core index): out = Dv^-1/2 Gc De^-1 Gc^T Dv^-1/2 (x W + 1 b^T),
computed in factored form (L never materialized), with the two big
contractions (over N and over E) in fp8-e4m3 DoubleRow matmuls (2 k-tiles
per instruction, ~2.4x bf16 PE throughput; 7e-3 rel err vs the 2e-2 gate):
  gs   = (16*dv) * gc           [N, E] fp8   (dv = rsqrt(rowsum gc))
  z^T  = x^T gs  per 128-row bf block        (MM1, fp8 DR over k-pairs)
  v    = z (W/16) + u0' (bias/16)^T [E, BF] fp8  (W-MM bf16; u0' = 16 u0)
  gsd  = (4096*de) * gc^T       [E, N] fp8
  out  = (dv/4096) * (gsd^T v)  [N, BF]      (MM2, fp8 DR over j-pairs)
The 16 / 4096 factors keep gs / gsd inside e4m3's normal range (dv ~ 0.04,
de ~ 1/512 are otherwise subnormal); they are exactly compensated by the
host-side W/16, bias/16 and the device-side dv/4096 evict scale.

Host-side (layout/dtype only): x, gc, gct packed partition-major and cast
to e4m3fn (halves load DMA); blockdiag(W,W)/16 bf16; bias/16 f32. Output
written partition-major bf16 and unpacked on host.

Scheduling: the early-DMA window crawls (~60 GB/s while all 8 cores start
simultaneously), so the two HWDGE rings lead with tiny gating pieces (gc
k0 + x k0 on sync, gc k1 + x k1 on scalar) and sweep 1 opens with two
plain single-k stages before switching to DoubleRow k-pairs; warmup
matmuls on a memset tile burn the PE p-state ramp during the wait. Sweep 1
runs m0-4 on parallel PSUM accumulators (the 6th bank stays free so dense
m12 starts the instant sweep 1 stops) with the padded-stationary stats
matmul interleaved. Dense sweeps run m12,m13 first, then m5-11, so the
final MM2 column-group's v-deps (m12-13) are ready mid-kernel; the five
MM2 groups (384x4+256 cols, aligned to m-tile triples) interleave between
sweeps at their WMM milestones. All stores ride the otherwise-idle sync
ring as consolidated half-group transfers (fine-grained for the last
group); psum evicts alternate DVE/ACT, split across both engines in the
drain-limited end phase; the stats/de/gsd chain avoids the Pool engine
(pathologically slow on tensor ops) and splits ACT/DVE instead.
"""

import sys

import numpy as np

sys.path.insert(0, "/opt/trn_rl_repo")

from contextlib import ExitStack

import ml_dtypes

import concourse.bass as bass
import concourse.mybir as mybir
import concourse.tile as tile
from concourse import bacc, bass_utils
from concourse.masks import make_identity

P = 128
T = 8
B = 28          # batch entries per core
N = 1024        # nodes
E = 512         # hyperedges (256 static + 256 dynamic)
F = 64          # features
BF = B * F      # 1792
EPS = 1e-6
NT = N // P     # 8 n-tiles
ET = E // P     # 4 e-tiles
MT = BF // P    # 14 bf-tiles (2 batch entries each)
KP = NT // 2    # 4 k-pairs for DoubleRow
M0 = 5          # m-tiles on parallel accumulators during the x load
                # (5, not 6: leaves one PSUM bank free so the first dense
                # sweep starts the instant sweep 1 stops)
SG = 16.0       # gs scale (fp8 range); compensated in host bdw, b2
SD = 4096.0     # gsd scale (fp8 range); compensated in evict dv/SD
N_WARM = 34     # full-width PE warmup matmuls during the load phase
N_WARM_FINE = 10  # tapered 64-col warmups at the end: coverage extends to
                  # the median gating-data arrival (~15.4us) while the
                  # handoff cost stays bounded by one ~90ns instruction
# MM2 column groups aligned to m-tile triples: deps g0:m0-2 .. g4:m12-13
GB = [0, 384, 768, 1152, 1536, 1792]

f32 = mybir.dt.float32
f32r = mybir.dt.float32r
bf16 = mybir.dt.bfloat16
f8 = mybir.dt.float8e4
BF16 = ml_dtypes.bfloat16
F8 = ml_dtypes.float8_e4m3fn
DR = mybir.MatmulPerfMode.DoubleRow


def _build_nc():
    nc = bacc.Bacc("TRN2", target_bir_lowering=False, debug=False)

    xs = nc.dram_tensor("xs", [P, NT * BF], f8, kind="ExternalInput").ap()
    gc = nc.dram_tensor("gc", [P, NT * E], f8, kind="ExternalInput").ap()
    gct = nc.dram_tensor("gct", [P, ET * N], f8, kind="ExternalInput").ap()
    bdw = nc.dram_tensor("bdw", [P, P], bf16, kind="ExternalInput").ap()
    b2 = nc.dram_tensor("b2", [1, P], f32, kind="ExternalInput").ap()
    os_ = nc.dram_tensor("os", [P, NT * BF], bf16, kind="ExternalOutput").ap()

    with tile.TileContext(nc) as tc, ExitStack() as ctx:
        const = ctx.enter_context(tc.tile_pool(name="const", bufs=1))
        big = ctx.enter_context(tc.tile_pool(name="big", bufs=1))
        ztp = ctx.enter_context(tc.tile_pool(name="ztp", bufs=8))
        osb = ctx.enter_context(tc.tile_pool(name="osb", bufs=3))
        ps_small = ctx.enter_context(tc.tile_pool(name="ps_small", bufs=2, space="PSUM"))
        ps_z = ctx.enter_context(tc.tile_pool(name="ps_z", bufs=2, space="PSUM"))
        ps_o = ctx.enter_context(tc.tile_pool(name="ps_o", bufs=2, space="PSUM"))
        ps_x = ctx.enter_context(tc.tile_pool(name="ps_x", bufs=2, space="PSUM"))

        # ---- input loads --------------------------------------------------
        gc_all = big.tile([P, NT, E], f8, name="gc_all")
        gc_r = gc.rearrange("p (k e) -> p k e", k=NT)
        xs_all = big.tile([P, NT, BF], f8, name="xs_all")
        xs_r = xs.rearrange("p (k m) -> p k m", k=NT)
        gct_all = big.tile([P, ET, N], f8, name="gct_all")
        gct_r = gct.rearrange("p (j n) -> p j n", j=ET)

        # Early DMA crawls (~60 GB/s while all 8 cores start up), so the
        # first pieces are TINY and the first two k-steps run as plain
        # single-k sweeps the moment each 64+80KB pair lands.
        # sync ring: k0 pair, gc k2-3, x kp1, m12-13, m5-11, gct j0-1
        nc.sync.dma_start(gc_all[:, 0:1], gc_r[:, 0:1])
        nc.sync.dma_start(xs_all[:, 0:1, 0 : M0 * P], xs_r[:, 0:1, 0 : M0 * P])
        nc.sync.dma_start(gc_all[:, 2:4], gc_r[:, 2:4])
        nc.sync.dma_start(xs_all[:, 2:4, 0 : M0 * P], xs_r[:, 2:4, 0 : M0 * P])
        nc.sync.dma_start(xs_all[:, 0:4, 12 * P : BF], xs_r[:, 0:4, 12 * P : BF])
        nc.sync.dma_start(xs_all[:, 0:4, M0 * P : 12 * P], xs_r[:, 0:4, M0 * P : 12 * P])
        nc.sync.dma_start(gct_all[:, 0:2], gct_r[:, 0:2])
        # scalar ring: k1 pair, gc k4-7, x kp2/kp3, m12-13, m5-11, gct j2-3
        nc.scalar.dma_start(gc_all[:, 1:2], gc_r[:, 1:2])
        nc.scalar.dma_start(xs_all[:, 1:2, 0 : M0 * P], xs_r[:, 1:2, 0 : M0 * P])
        nc.scalar.dma_start(gc_all[:, 4:6], gc_r[:, 4:6])
        nc.scalar.dma_start(xs_all[:, 4:6, 0 : M0 * P], xs_r[:, 4:6, 0 : M0 * P])
        nc.scalar.dma_start(gc_all[:, 6:8], gc_r[:, 6:8])
        nc.scalar.dma_start(xs_all[:, 6:8, 0 : M0 * P], xs_r[:, 6:8, 0 : M0 * P])
        nc.scalar.dma_start(xs_all[:, 4:8, 12 * P : BF], xs_r[:, 4:8, 12 * P : BF])
        nc.scalar.dma_start(xs_all[:, 4:8, M0 * P : 12 * P], xs_r[:, 4:8, M0 * P : 12 * P])
        nc.scalar.dma_start(gct_all[:, 2:4], gct_r[:, 2:4])
        bdw_sb = const.tile([P, P], bf16, name="bdw_sb")
        nc.gpsimd.dma_start(bdw_sb[:], bdw)
        b2_sb = const.tile([1, P], f32, name="b2_sb")
        nc.gpsimd.dma_start(b2_sb[:], b2)

        bias_bc = const.tile([P, P], f32, name="bias_bc")
        nc.gpsimd.partition_broadcast(bias_bc[:], b2_sb[:])

        ident_f = const.tile([P, P], f32, name="ident_f")
        make_identity(nc, ident_f[:])
        ident = const.tile([P, P], f32r, name="ident")
        nc.vector.tensor_copy(ident[:], ident_f[:])

        # ---- PE warmup: burn the p-state ramp during the load phase ------
        warm_src = const.tile([P, 256], bf16, name="warm_src")
        nc.vector.memset(warm_src[:], 1.0)
        warm_ps = ps_small.tile([P, ET, P], f32, name="sp")
        warm_out = warm_ps[:, 0:2, :].rearrange("p a b -> p (a b)")
        for _ in range(N_WARM):
            nc.tensor.matmul(
                warm_out, warm_src[:, 0:P], warm_src[:],
                start=True, stop=True,
            )
        for _ in range(N_WARM_FINE):
            nc.tensor.matmul(
                warm_out[:, 0:64], warm_src[:, 0:P], warm_src[:, 0:64],
                start=True, stop=True,
            )

        # ---- per-k dv chain + gs scale (unblocks MM1 k-pair by k-pair) ---
        eps_col = const.tile([P, 1], f32, name="eps_col")
        rs = const.tile([P, NT, 1], f32, name="rs")
        sq = const.tile([P, NT], f32, name="sq")
        dv = const.tile([P, NT], f32, name="dv")
        dvd = const.tile([P, NT], f32, name="dvd")
        gs_all = big.tile([P, NT, E], f8, name="gs_all")
        trash = const.tile([P, E], bf16, name="trash")
        # stats stationary: [ones | 16*dv | 0-pad to 32] per k-tile (fp8);
        # the pad makes the DoubleRow ISA check accept it
        onesdv = const.tile([P, NT, 32], f8, name="onesdv")
        with tc.high_priority():
            nc.vector.memset(eps_col[:], EPS)
            nc.vector.memset(onesdv[:], 0.0)
            nc.vector.memset(onesdv[:, :, 0:1], 1.0)
            for k in range(NT):
                # rowsum on ACT via copy with accum_out (scratch output);
                # DVE does recip + fp8 requant only
                nc.scalar.activation(
                    trash[:], gc_all[:, k, :],
                    mybir.ActivationFunctionType.Copy,
                    accum_out=rs[:, k],
                )
                nc.scalar.activation(
                    sq[:, k : k + 1], rs[:, k], mybir.ActivationFunctionType.Sqrt,
                    bias=eps_col[:],
                )
                nc.vector.reciprocal(dv[:, k : k + 1], sq[:, k : k + 1])
                nc.vector.tensor_scalar(
                    out=gs_all[:, k, :], in0=gc_all[:, k, :],
                    scalar1=dv[:, k : k + 1],
                    scalar2=SG, op0=mybir.AluOpType.mult,
                    op1=mybir.AluOpType.mult,
                )
                nc.scalar.mul(onesdv[:, k, 1:2], dv[:, k : k + 1], SG)
            # evict scale for MM2 (gates nothing early)
            nc.vector.tensor_scalar(
                out=dvd[:], in0=dv[:], scalar1=1.0 / SD, scalar2=None,
                op0=mybir.AluOpType.mult,
            )

        v_all = big.tile([P, ET, BF], f8, name="v_all")
        stats_tile = ps_small.tile([P, ET, P], f32, name="sp")
        stats_ps = stats_tile[0:2].rearrange("p a b -> p (a b)")
        # 32-row matmul view of the same bank (rows 2-31 accumulate zeros
        # from the onesdv pad)
        stats_out = stats_tile[0:32].rearrange("p a b -> p (a b)")
        stats_sb = const.tile([2, E], f32r, name="stats_sb")
        statsT = const.tile([P, ET, 2], f32, name="statsT")
        de_col = const.tile([P, ET], f32, name="de_col")
        gsd_all = big.tile([P, ET, N], f8, name="gsd_all")
        bias_u0 = const.tile([P, ET, P], f32, name="bias_u0")

        def emit_stats_tail():
            # stats_ps rows [colsum(Gc) | 16*colsum(Gs)] accumulated during
            # sweep 1; transpose to columns. Copies ride DVE so the ACT
            # queue stays clear for the zt evicts gating the dense sweeps.
            nc.vector.tensor_copy(stats_sb[:], stats_ps)
            for j in range(ET):
                tp = ps_small.tile([P, ET, P], f32r, name="sp")[:, 0, 0:2]
                nc.tensor.matmul(
                    tp, stats_sb[:, j * P : (j + 1) * P], ident[0:2, 0:2],
                    is_transpose=True,
                )
                nc.vector.tensor_copy(statsT[:, j, :], tp)
            nc.vector.tensor_scalar(
                out=de_col[:], in0=statsT[:, :, 0], scalar1=EPS, scalar2=None,
                op0=mybir.AluOpType.add,
            )
            nc.vector.reciprocal(de_col[:], de_col[:])
            # fold the SD gsd-scale into de itself (de_col := SD/colsum)
            nc.vector.tensor_scalar(
                out=de_col[:], in0=de_col[:], scalar1=SD, scalar2=None,
                op0=mybir.AluOpType.mult,
            )
            # bias_u0 = u0' (x) bias/16 on DVE (fast, gates all v-STTs);
            # gsd = de * Gc^T (fp8): j0-1 on ACT, j2-3 on Pool
            for j in range(ET):
                nc.vector.tensor_scalar(
                    out=bias_u0[:, j, :], in0=bias_bc[:],
                    scalar1=statsT[:, j, 1:2], scalar2=None,
                    op0=mybir.AluOpType.mult,
                )
            # Pool is pathologically slow on these (14us observed) — keep
            # gsd off it: j0-1 on ACT, j2-3 on DVE
            for j in range(2):
                nc.scalar.mul(
                    gsd_all[:, j, :], gct_all[:, j, :], de_col[:, j : j + 1]
                )
            for j in range(2, ET):
                nc.vector.tensor_scalar(
                    out=gsd_all[:, j, :], in0=gct_all[:, j, :],
                    scalar1=de_col[:, j : j + 1], scalar2=None,
                    op0=mybir.AluOpType.mult,
                )

        def emit_wmm_from_zt(m, zt):
            wps = ps_small.tile([P, ET, P], f32, name="sp")
            for j in range(ET):
                nc.tensor.matmul(
                    wps[:, j, :], zt[:, j * P : (j + 1) * P], bdw_sb[:],
                    start=True, stop=True,
                )
            # v = bias_u0 + zw, rounded to fp8 (one DVE instr per m-tile)
            nc.vector.scalar_tensor_tensor(
                out=v_all[:, :, m * P : (m + 1) * P],
                in0=bias_u0[:],
                scalar=1.0,
                in1=wps[:],
                op0=mybir.AluOpType.mult,
                op1=mybir.AluOpType.add,
            )

        def emit_wmm(m, zps, act_evict):
            zt = ztp.tile([P, E], bf16, name="zt")
            if act_evict:
                nc.scalar.copy(zt[:], zps[:])
            else:
                nc.vector.tensor_copy(zt[:], zps[:])
            emit_wmm_from_zt(m, zt)

        # ---- MM1 sweep 1: m0-5 on parallel accumulators, kp-by-kp --------
        zpools = [ps_z, ps_o, ps_x]
        zps4 = [
            zpools[m // 2].tile([P, E], f32, name="zps") for m in range(M0)
        ]
        # stages: k0 plain, k1 plain (each gated on a tiny 144KB pair so PE
        # starts ~10us), then k-pairs 1-3 as DoubleRow
        stages = [(0, None), (1, None)] + [(None, kp) for kp in range(1, KP)]
        for si, (k1, kp) in enumerate(stages):
            first, lastst = si == 0, si == len(stages) - 1
            if kp is None:
                for m in range(M0):
                    nc.tensor.matmul(
                        zps4[m][:], xs_all[:, k1, m * P : (m + 1) * P],
                        gs_all[:, k1, :],
                        start=first, stop=lastst,
                    )
                kks = (k1,)
            else:
                ks = slice(2 * kp, 2 * kp + 2)
                for m in range(M0):
                    nc.tensor.matmul(
                        zps4[m][:], xs_all[:, ks, m * P : (m + 1) * P],
                        gs_all[:, ks, :],
                        start=first, stop=lastst, perf_mode=DR,
                    )
                nc.tensor.matmul(
                    stats_out, onesdv[:, ks, :], gc_all[:, ks, :],
                    start=False, stop=lastst, perf_mode=DR,
                    skip_group_check=True,
                )
                kks = ()
            for kk in kks:
                nc.tensor.matmul(
                    stats_out, onesdv[:, kk, :], gc_all[:, kk, :],
                    start=(kk == 0), stop=False, skip_group_check=True,
                )
        # evict sweep-1 psums (alternate engines) to free banks; the stats
        # chain is emitted interleaved below (dense m12 first keeps PE busy)

        # ---- dense sweeps + W-MMs + MM2 column groups --------------------
        os_r = os_.rearrange("p (i m) -> p i m", i=NT)

        def dense(m, pool):
            zps = pool.tile([P, E], f32, name="zps")
            for kp in range(KP):
                ks = slice(2 * kp, 2 * kp + 2)
                nc.tensor.matmul(
                    zps[:], xs_all[:, ks, m * P : (m + 1) * P],
                    gs_all[:, ks, :],
                    start=(kp == 0), stop=(kp == KP - 1), perf_mode=DR,
                )
            return zps

        def emit_mm2_group(g, last=False, split_evict=False):
            c0, c1 = GB[g], GB[g + 1]
            W = c1 - c0
            ost = osb.tile([P, NT, W], bf16, name="ost")
            for i in range(NT):
                ops = zpools[(g + i) % 3].tile([P, E], f32, name="zps")[:, 0:W]
                for jp in range(ET // 2):
                    js = slice(2 * jp, 2 * jp + 2)
                    nc.tensor.matmul(
                        ops[:], gsd_all[:, js, i * P : (i + 1) * P],
                        v_all[:, js, c0:c1],
                        start=(jp == 0), stop=(jp == ET // 2 - 1), perf_mode=DR,
                    )
                if split_evict:
                    # drain-limited end phase: halve psum hold time by
                    # evicting each chunk on both engines at once
                    h = W // 2
                    nc.vector.tensor_scalar(
                        out=ost[:, i, 0:h], in0=ops[:, 0:h],
                        scalar1=dvd[:, i : i + 1],
                        scalar2=None, op0=mybir.AluOpType.mult,
                    )
                    nc.scalar.mul(ost[:, i, h:W], ops[:, h:W], dvd[:, i : i + 1])
                elif i % 2 == 0:
                    nc.vector.tensor_scalar(
                        out=ost[:, i, :], in0=ops[:], scalar1=dvd[:, i : i + 1],
                        scalar2=None, op0=mybir.AluOpType.mult,
                    )
                else:
                    nc.scalar.mul(ost[:, i, :], ops[:], dvd[:, i : i + 1])
                if last:
                    # fine-grained stores to shrink the serial tail; all
                    # stores ride the sync ring (SP is otherwise idle)
                    if i % 2 == 1:
                        nc.sync.dma_start(
                            os_r[:, i - 1 : i + 1, c0:c1], ost[:, i - 1 : i + 1]
                        )
                elif i == NT // 2 - 1:
                    nc.sync.dma_start(os_r[:, 0:4, c0:c1], ost[:, 0:4])
            if not last:
                nc.sync.dma_start(os_r[:, 4:8, c0:c1], ost[:, 4:8])

        # sweep-1 banks: zps4[m0-4] = z.b0, z.b1, o.b0, o.b1, x.b0; x.b1 is
        # free, so dense m12 (ps_x) starts the instant sweep 1 stops. Each
        # later dense call's pool is chosen so its bank was evicted longest
        # ago. The stats chain + zt4 evicts queue on ACT/DVE underneath.
        z12 = dense(12, ps_x)
        emit_stats_tail()
        zt4 = []
        for m in range(M0):
            zt = ztp.tile([P, E], bf16, name="zt")
            if m % 2 == 0:
                nc.scalar.copy(zt[:], zps4[m][:])
            else:
                nc.vector.tensor_copy(zt[:], zps4[m][:])
            zt4.append(zt)
        z13 = dense(13, ps_z)
        z5 = dense(5, ps_z)
        z6 = dense(6, ps_o)
        emit_wmm(12, z12, act_evict=True)
        emit_wmm_from_zt(0, zt4[0])
        emit_wmm(13, z13, act_evict=False)
        emit_wmm_from_zt(1, zt4[1])
        z7 = dense(7, ps_o)
        emit_wmm(5, z5, act_evict=True)
        emit_wmm_from_zt(2, zt4[2])
        z8 = dense(8, ps_x)
        emit_wmm(6, z6, act_evict=False)
        emit_wmm_from_zt(3, zt4[3])
        z9 = dense(9, ps_x)
        emit_wmm(7, z7, act_evict=True)
        emit_wmm_from_zt(4, zt4[4])
        emit_mm2_group(0, split_evict=True)
        z10 = dense(10, ps_z)
        emit_wmm(8, z8, act_evict=False)
        emit_mm2_group(1, split_evict=True)
        z11 = dense(11, ps_z)
        emit_wmm(9, z9, act_evict=True)
        emit_wmm(10, z10, act_evict=False)
        emit_wmm(11, z11, act_evict=True)
        emit_mm2_group(2, split_evict=True)
        emit_mm2_group(3, split_evict=True)
        emit_mm2_group(4, last=True, split_evict=True)

    nc.finalize()
    return nc


_NC = None


def _get_nc():
    global _NC
    if _NC is None:
        _NC = _build_nc()
    return _NC


def _in_maps(x, G, G1, weight, bias):
    x = np.ascontiguousarray(x, dtype=np.float32)
    G = np.ascontiguousarray(G, dtype=np.float32)
    G1 = np.ascontiguousarray(G1, dtype=np.float32)
    weight = np.ascontiguousarray(weight, dtype=np.float32)
    bias = np.ascontiguousarray(bias, dtype=np.float32)

    # x[t,b,n,f] -> packed [T, P, (k b f)]: partition row p holds the
    # k-tile-major concat of x[t, :, k*128+p, :] (one contiguous HBM run)
    xh = np.ascontiguousarray(
        x.reshape(T, B, NT, P, F).transpose(0, 3, 2, 1, 4)
    ).reshape(T, P, NT * BF).astype(F8)
    # Gc = [G | G1[t]] packed as [T, P, (k e)]; transpose as [T, P, (j n)]
    gc_np = np.concatenate(
        [np.broadcast_to(G[None], (T, N, 256)), G1], axis=2
    )
    gch = np.ascontiguousarray(
        gc_np.reshape(T, NT, P, E).transpose(0, 2, 1, 3)
    ).reshape(T, P, NT * E).astype(F8)
    gcth = np.ascontiguousarray(
        gc_np.transpose(0, 2, 1).reshape(T, ET, P, N).transpose(0, 2, 1, 3)
    ).reshape(T, P, ET * N).astype(F8)
    # blockdiag(W, W)/SG built on host (compensates the 16x in gs)
    bdw_h = np.zeros((P, P), dtype=BF16)
    bdw_h[:F, :F] = (weight / SG).astype(BF16)
    bdw_h[F:, F:] = (weight / SG).astype(BF16)
    b2_h = (np.tile(bias, 2) / SG).reshape(1, P).astype(np.float32)

    maps = []
    for c in range(T):
        maps.append(
            {
                "xs": xh[c],
                "gc": gch[c],
                "gct": gcth[c],
                "bdw": bdw_h,
                "b2": b2_h,
            }
        )
    return maps


def kernel(x, G, G1, weight, bias):
    nc = _get_nc()
    res = bass_utils.run_bass_kernel_spmd(
        nc, _in_maps(x, G, G1, weight, bias), core_ids=list(range(T))
    )
    # os: per core [P, (i b f)] bf16 -> out[b, i*128+p, f] f32
    out = np.stack([np.asarray(r["os"]) for r in res.results], axis=0)
    return np.ascontiguousarray(
        out.reshape(T, P, NT, B, F).transpose(0, 3, 2, 1, 4)
    ).reshape(T * B, N, F).astype(np.float32)


# revision 38
# speedup vs baseline: 1.0022x; 1.0022x over previous
"""HGNN conv kernel for Trainium2, data-parallel over time across 8 cores.

Per core (t = core index): out = Dv^-1/2 Gc De^-1 Gc^T Dv^-1/2 (x W + 1 b^T),
computed in factored form (L never materialized), with the two big
contractions (over N and over E) in fp8-e4m3 DoubleRow matmuls (2 k-tiles
per instruction, ~2.4x bf16 PE throughput; 7e-3 rel err vs the 2e-2 gate):
  gs   = (16*dv) * gc           [N, E] fp8   (dv = rsqrt(rowsum gc))
  z^T  = x^T gs  per 128-row bf block        (MM1, fp8 DR over k-pairs)
  v    = z (W/16) + u0' (bias/16)^T [E, BF] fp8  (W-MM bf16; u0' = 16 u0)
  gsd  = (4096*de) * gc^T       [E, N] fp8
  out  = (dv/4096) * (gsd^T v)  [N, BF]      (MM2, fp8 DR over j-pairs)
The 16 / 4096 factors keep gs / gsd inside e4m3's normal range (dv ~ 0.04,
de ~ 1/512 are otherwise subnormal); they are exactly compensated by the
host-side W/16, bias/16 and the device-side dv/4096 evict scale.

Host-side (layout/dtype only): x, gc, gct packed partition-major and cast
to e4m3fn (halves load DMA); blockdiag(W,W)/16 bf16; bias/16 f32. Output
written partition-major bf16 and unpacked on host.

Scheduling: the early-DMA window crawls (~60 GB/s while all 8 cores start
simultaneously), so the two HWDGE rings lead with tiny gating pieces (gc
k0 + x k0 on sync, gc k1 + x k1 on scalar) and sweep 1 opens with two
plain single-k stages before switching to DoubleRow k-pairs; warmup
matmuls on a memset tile burn the PE p-state ramp during the wait. Sweep 1
runs m0-4 on parallel PSUM accumulators (the 6th bank stays free so dense
m12 starts the instant sweep 1 stops) with the padded-stationary stats
matmul interleaved. Dense sweeps run m12,m13 first, then m5-11, so the
final MM2 column-group's v-deps (m12-13) are ready mid-kernel; the five
MM2 groups (384x4+256 cols, aligned to m-tile triples) interleave between
sweeps at their WMM milestones. All stores ride the otherwise-idle sync
ring as consolidated half-group transfers (fine-grained for the last
group); psum evicts alternate DVE/ACT, split across both engines in the
drain-limited end phase; the stats/de/gsd chain avoids the Pool engine
(pathologically slow on tensor ops) and splits ACT/DVE instead.
"""

import sys

import numpy as np

sys.path.insert(0, "/opt/trn_rl_repo")

from contextlib import ExitStack

import ml_dtypes

import concourse.bass as bass
import concourse.mybir as mybir
import concourse.tile as tile
from concourse import bacc, bass_utils
from concourse.masks import make_identity

P = 128
T = 8
B = 28          # batch entries per core
N = 1024        # nodes
E = 512         # hyperedges (256 static + 256 dynamic)
F = 64          # features
BF = B * F      # 1792
EPS = 1e-6
NT = N // P     # 8 n-tiles
ET = E // P     # 4 e-tiles
MT = BF // P    # 14 bf-tiles (2 batch entries each)
KP = NT // 2    # 4 k-pairs for DoubleRow
M0 = 5          # m-tiles on parallel accumulators during the x load
                # (5, not 6: leaves one PSUM bank free so the first dense
                # sweep starts the instant sweep 1 stops)
SG = 16.0       # gs scale (fp8 range); compensated in host bdw, b2
SD = 4096.0     # gsd scale (fp8 range); compensated in evict dv/SD
N_WARM = 34     # full-width PE warmup matmuls during the load phase
N_WARM_FINE = 18  # tapered 64-col warmups at the end: coverage extends to
                  # the median gating-data arrival (~15.4us) while the
                  # handoff cost stays bounded by one ~90ns instruction
# MM2 column groups aligned to m-tile triples: deps g0:m0-2 .. g4:m12-13
GB = [0, 384, 768, 1152, 1536, 1792]

f32 = mybir.dt.float32
f32r = mybir.dt.float32r
bf16 = mybir.dt.bfloat16
f8 = mybir.dt.float8e4
BF16 = ml_dtypes.bfloat16
F8 = ml_dtypes.float8_e4m3fn
DR = mybir.MatmulPerfMode.DoubleRow


def _build_nc():
    nc = bacc.Bacc("TRN2", target_bir_lowering=False, debug=False)

    xs = nc.dram_tensor("xs", [P, NT * BF], f8, kind="ExternalInput").ap()
    gc = nc.dram_tensor("gc", [P, NT * E], f8, kind="ExternalInput").ap()
    gct = nc.dram_tensor("gct", [P, ET * N], f8, kind="ExternalInput").ap()
    bdw = nc.dram_tensor("bdw", [P, P], bf16, kind="ExternalInput").ap()
    b2 = nc.dram_tensor("b2", [1, P], f32, kind="ExternalInput").ap()
    os_ = nc.dram_tensor("os", [P, NT * BF], bf16, kind="ExternalOutput").ap()

    with tile.TileContext(nc) as tc, ExitStack() as ctx:
        const = ctx.enter_context(tc.tile_pool(name="const", bufs=1))
        big = ctx.enter_context(tc.tile_pool(name="big", bufs=1))
        ztp = ctx.enter_context(tc.tile_pool(name="ztp", bufs=8))
        osb = ctx.enter_context(tc.tile_pool(name="osb", bufs=3))
        ps_small = ctx.enter_context(tc.tile_pool(name="ps_small", bufs=2, space="PSUM"))
        ps_z = ctx.enter_context(tc.tile_pool(name="ps_z", bufs=2, space="PSUM"))
        ps_o = ctx.enter_context(tc.tile_pool(name="ps_o", bufs=2, space="PSUM"))
        ps_x = ctx.enter_context(tc.tile_pool(name="ps_x", bufs=2, space="PSUM"))

        # ---- input loads --------------------------------------------------
        gc_all = big.tile([P, NT, E], f8, name="gc_all")
        gc_r = gc.rearrange("p (k e) -> p k e", k=NT)
        xs_all = big.tile([P, NT, BF], f8, name="xs_all")
        xs_r = xs.rearrange("p (k m) -> p k m", k=NT)
        gct_all = big.tile([P, ET, N], f8, name="gct_all")
        gct_r = gct.rearrange("p (j n) -> p j n", j=ET)

        # Early DMA crawls (~60 GB/s while all 8 cores start up), so the
        # first pieces are TINY and the first two k-steps run as plain
        # single-k sweeps the moment each 64+80KB pair lands.
        # sync ring: k0 pair, gc k2-3, x kp1, m12-13, m5-11, gct j0-1
        nc.sync.dma_start(gc_all[:, 0:1], gc_r[:, 0:1])
        nc.sync.dma_start(xs_all[:, 0:1, 0 : M0 * P], xs_r[:, 0:1, 0 : M0 * P])
        nc.sync.dma_start(gc_all[:, 2:4], gc_r[:, 2:4])
        nc.sync.dma_start(xs_all[:, 2:4, 0 : M0 * P], xs_r[:, 2:4, 0 : M0 * P])
        nc.sync.dma_start(xs_all[:, 0:4, 12 * P : BF], xs_r[:, 0:4, 12 * P : BF])
        nc.sync.dma_start(xs_all[:, 0:4, M0 * P : 12 * P], xs_r[:, 0:4, M0 * P : 12 * P])
        nc.sync.dma_start(gct_all[:, 0:2], gct_r[:, 0:2])
        # scalar ring: k1 pair, gc k4-7, x kp2/kp3, m12-13, m5-11, gct j2-3
        nc.scalar.dma_start(gc_all[:, 1:2], gc_r[:, 1:2])
        nc.scalar.dma_start(xs_all[:, 1:2, 0 : M0 * P], xs_r[:, 1:2, 0 : M0 * P])
        nc.scalar.dma_start(gc_all[:, 4:6], gc_r[:, 4:6])
        nc.scalar.dma_start(xs_all[:, 4:6, 0 : M0 * P], xs_r[:, 4:6, 0 : M0 * P])
        nc.scalar.dma_start(gc_all[:, 6:8], gc_r[:, 6:8])
        nc.scalar.dma_start(xs_all[:, 6:8, 0 : M0 * P], xs_r[:, 6:8, 0 : M0 * P])
        nc.scalar.dma_start(xs_all[:, 4:8, 12 * P : BF], xs_r[:, 4:8, 12 * P : BF])
        nc.scalar.dma_start(xs_all[:, 4:8, M0 * P : 12 * P], xs_r[:, 4:8, M0 * P : 12 * P])
        nc.scalar.dma_start(gct_all[:, 2:4], gct_r[:, 2:4])
        bdw_sb = const.tile([P, P], bf16, name="bdw_sb")
        nc.gpsimd.dma_start(bdw_sb[:], bdw)
        b2_sb = const.tile([1, P], f32, name="b2_sb")
        nc.gpsimd.dma_start(b2_sb[:], b2)

        bias_bc = const.tile([P, P], f32, name="bias_bc")
        nc.gpsimd.partition_broadcast(bias_bc[:], b2_sb[:])

        ident_f = const.tile([P, P], f32, name="ident_f")
        make_identity(nc, ident_f[:])
        ident = const.tile([P, P], f32r, name="ident")
        nc.vector.tensor_copy(ident[:], ident_f[:])

        # ---- PE warmup: burn the p-state ramp during the load phase ------
        warm_src = const.tile([P, 256], bf16, name="warm_src")
        nc.vector.memset(warm_src[:], 1.0)
        warm_ps = ps_small.tile([P, ET, P], f32, name="sp")
        warm_out = warm_ps[:, 0:2, :].rearrange("p a b -> p (a b)")
        for _ in range(N_WARM):
            nc.tensor.matmul(
                warm_out, warm_src[:, 0:P], warm_src[:],
                start=True, stop=True,
            )
        for _ in range(N_WARM_FINE):
            nc.tensor.matmul(
                warm_out[:, 0:64], warm_src[:, 0:P], warm_src[:, 0:64],
                start=True, stop=True,
            )

        # ---- per-k dv chain + gs scale (unblocks MM1 k-pair by k-pair) ---
        eps_col = const.tile([P, 1], f32, name="eps_col")
        rs = const.tile([P, NT, 1], f32, name="rs")
        sq = const.tile([P, NT], f32, name="sq")
        dv = const.tile([P, NT], f32, name="dv")
        dvd = const.tile([P, NT], f32, name="dvd")
        gs_all = big.tile([P, NT, E], f8, name="gs_all")
        trash = const.tile([P, E], bf16, name="trash")
        # stats stationary: [ones | 16*dv | 0-pad to 32] per k-tile (fp8);
        # the pad makes the DoubleRow ISA check accept it
        onesdv = const.tile([P, NT, 32], f8, name="onesdv")
        with tc.high_priority():
            nc.vector.memset(eps_col[:], EPS)
            nc.vector.memset(onesdv[:], 0.0)
            nc.vector.memset(onesdv[:, :, 0:1], 1.0)
            for k in range(NT):
                # rowsum on ACT via copy with accum_out (scratch output);
                # DVE does recip + fp8 requant only
                nc.scalar.activation(
                    trash[:], gc_all[:, k, :],
                    mybir.ActivationFunctionType.Copy,
                    accum_out=rs[:, k],
                )
                nc.scalar.activation(
                    sq[:, k : k + 1], rs[:, k], mybir.ActivationFunctionType.Sqrt,
                    bias=eps_col[:],
                )
                nc.vector.reciprocal(dv[:, k : k + 1], sq[:, k : k + 1])
                nc.vector.tensor_scalar(
                    out=gs_all[:, k, :], in0=gc_all[:, k, :],
                    scalar1=dv[:, k : k + 1],
                    scalar2=SG, op0=mybir.AluOpType.mult,
                    op1=mybir.AluOpType.mult,
                )
                nc.scalar.mul(onesdv[:, k, 1:2], dv[:, k : k + 1], SG)
            # evict scale for MM2 (gates nothing early)
            nc.vector.tensor_scalar(
                out=dvd[:], in0=dv[:], scalar1=1.0 / SD, scalar2=None,
                op0=mybir.AluOpType.mult,
            )

        v_all = big.tile([P, ET, BF], f8, name="v_all")
        stats_tile = ps_small.tile([P, ET, P], f32, name="sp")
        stats_ps = stats_tile[0:2].rearrange("p a b -> p (a b)")
        # 32-row matmul view of the same bank (rows 2-31 accumulate zeros
        # from the onesdv pad)
        stats_out = stats_tile[0:32].rearrange("p a b -> p (a b)")
        stats_sb = const.tile([2, E], f32r, name="stats_sb")
        statsT = const.tile([P, ET, 2], f32, name="statsT")
        de_col = const.tile([P, ET], f32, name="de_col")
        gsd_all = big.tile([P, ET, N], f8, name="gsd_all")
        bias_u0 = const.tile([P, ET, P], f32, name="bias_u0")

        def emit_stats_tail():
            # stats_ps rows [colsum(Gc) | 16*colsum(Gs)] accumulated during
            # sweep 1; transpose to columns. Copies ride DVE so the ACT
            # queue stays clear for the zt evicts gating the dense sweeps.
            nc.vector.tensor_copy(stats_sb[:], stats_ps)
            for j in range(ET):
                tp = ps_small.tile([P, ET, P], f32r, name="sp")[:, 0, 0:2]
                nc.tensor.matmul(
                    tp, stats_sb[:, j * P : (j + 1) * P], ident[0:2, 0:2],
                    is_transpose=True,
                )
                nc.vector.tensor_copy(statsT[:, j, :], tp)
            nc.vector.tensor_scalar(
                out=de_col[:], in0=statsT[:, :, 0], scalar1=EPS, scalar2=None,
                op0=mybir.AluOpType.add,
            )
            nc.vector.reciprocal(de_col[:], de_col[:])
            # fold the SD gsd-scale into de itself (de_col := SD/colsum)
            nc.vector.tensor_scalar(
                out=de_col[:], in0=de_col[:], scalar1=SD, scalar2=None,
                op0=mybir.AluOpType.mult,
            )
            # bias_u0 = u0' (x) bias/16 on DVE (fast, gates all v-STTs);
            # gsd = de * Gc^T (fp8): j0-1 on ACT, j2-3 on Pool
            for j in range(ET):
                nc.vector.tensor_scalar(
                    out=bias_u0[:, j, :], in0=bias_bc[:],
                    scalar1=statsT[:, j, 1:2], scalar2=None,
                    op0=mybir.AluOpType.mult,
                )
            # Pool is pathologically slow on these (14us observed) — keep
            # gsd off it: j0-1 on ACT, j2-3 on DVE
            for j in range(2):
                nc.scalar.mul(
                    gsd_all[:, j, :], gct_all[:, j, :], de_col[:, j : j + 1]
                )
            for j in range(2, ET):
                nc.vector.tensor_scalar(
                    out=gsd_all[:, j, :], in0=gct_all[:, j, :],
                    scalar1=de_col[:, j : j + 1], scalar2=None,
                    op0=mybir.AluOpType.mult,
                )

        def emit_wmm_from_zt(m, zt):
            wps = ps_small.tile([P, ET, P], f32, name="sp")
            for j in range(ET):
                nc.tensor.matmul(
                    wps[:, j, :], zt[:, j * P : (j + 1) * P], bdw_sb[:],
                    start=True, stop=True,
                )
            # v = bias_u0 + zw, rounded to fp8 (one DVE instr per m-tile)
            nc.vector.scalar_tensor_tensor(
                out=v_all[:, :, m * P : (m + 1) * P],
                in0=bias_u0[:],
                scalar=1.0,
                in1=wps[:],
                op0=mybir.AluOpType.mult,
                op1=mybir.AluOpType.add,
            )

        def emit_wmm(m, zps, act_evict):
            zt = ztp.tile([P, E], bf16, name="zt")
            if act_evict:
                nc.scalar.copy(zt[:], zps[:])
            else:
                nc.vector.tensor_copy(zt[:], zps[:])
            emit_wmm_from_zt(m, zt)

        # ---- MM1 sweep 1: m0-5 on parallel accumulators, kp-by-kp --------
        zpools = [ps_z, ps_o, ps_x]
        zps4 = [
            zpools[m // 2].tile([P, E], f32, name="zps") for m in range(M0)
        ]
        # stages: k0 plain, k1 plain (each gated on a tiny 144KB pair so PE
        # starts ~10us), then k-pairs 1-3 as DoubleRow
        stages = [(0, None), (1, None)] + [(None, kp) for kp in range(1, KP)]
        for si, (k1, kp) in enumerate(stages):
            first, lastst = si == 0, si == len(stages) - 1
            if kp is None:
                for m in range(M0):
                    nc.tensor.matmul(
                        zps4[m][:], xs_all[:, k1, m * P : (m + 1) * P],
                        gs_all[:, k1, :],
                        start=first, stop=lastst,
                    )
                kks = (k1,)
            else:
                ks = slice(2 * kp, 2 * kp + 2)
                for m in range(M0):
                    nc.tensor.matmul(
                        zps4[m][:], xs_all[:, ks, m * P : (m + 1) * P],
                        gs_all[:, ks, :],
                        start=first, stop=lastst, perf_mode=DR,
                    )
                nc.tensor.matmul(
                    stats_out, onesdv[:, ks, :], gc_all[:, ks, :],
                    start=False, stop=lastst, perf_mode=DR,
                    skip_group_check=True,
                )
                kks = ()
            for kk in kks:
                nc.tensor.matmul(
                    stats_out, onesdv[:, kk, :], gc_all[:, kk, :],
                    start=(kk == 0), stop=False, skip_group_check=True,
                )
        # evict sweep-1 psums (alternate engines) to free banks; the stats
        # chain is emitted interleaved below (dense m12 first keeps PE busy)

        # ---- dense sweeps + W-MMs + MM2 column groups --------------------
        os_r = os_.rearrange("p (i m) -> p i m", i=NT)

        def dense(m, pool):
            zps = pool.tile([P, E], f32, name="zps")
            for kp in range(KP):
                ks = slice(2 * kp, 2 * kp + 2)
                nc.tensor.matmul(
                    zps[:], xs_all[:, ks, m * P : (m + 1) * P],
                    gs_all[:, ks, :],
                    start=(kp == 0), stop=(kp == KP - 1), perf_mode=DR,
                )
            return zps

        def emit_mm2_group(g, last=False, split_evict=False):
            c0, c1 = GB[g], GB[g + 1]
            W = c1 - c0
            ost = osb.tile([P, NT, W], bf16, name="ost")
            for i in range(NT):
                ops = zpools[(g + i) % 3].tile([P, E], f32, name="zps")[:, 0:W]
                for jp in range(ET // 2):
                    js = slice(2 * jp, 2 * jp + 2)
                    nc.tensor.matmul(
                        ops[:], gsd_all[:, js, i * P : (i + 1) * P],
                        v_all[:, js, c0:c1],
                        start=(jp == 0), stop=(jp == ET // 2 - 1), perf_mode=DR,
                    )
                if split_evict:
                    # drain-limited end phase: halve psum hold time by
                    # evicting each chunk on both engines at once
                    h = W // 2
                    nc.vector.tensor_scalar(
                        out=ost[:, i, 0:h], in0=ops[:, 0:h],
                        scalar1=dvd[:, i : i + 1],
                        scalar2=None, op0=mybir.AluOpType.mult,
                    )
                    nc.scalar.mul(ost[:, i, h:W], ops[:, h:W], dvd[:, i : i + 1])
                elif i % 2 == 0:
                    nc.vector.tensor_scalar(
                        out=ost[:, i, :], in0=ops[:], scalar1=dvd[:, i : i + 1],
                        scalar2=None, op0=mybir.AluOpType.mult,
                    )
                else:
                    nc.scalar.mul(ost[:, i, :], ops[:], dvd[:, i : i + 1])
                if last:
                    # fine-grained stores to shrink the serial tail; all
                    # stores ride the sync ring (SP is otherwise idle)
                    if i % 2 == 1:
                        nc.sync.dma_start(
                            os_r[:, i - 1 : i + 1, c0:c1], ost[:, i - 1 : i + 1]
                        )
                elif i == NT // 2 - 1:
                    nc.sync.dma_start(os_r[:, 0:4, c0:c1], ost[:, 0:4])
            if not last:
                nc.sync.dma_start(os_r[:, 4:8, c0:c1], ost[:, 4:8])

        # sweep-1 banks: zps4[m0-4] = z.b0, z.b1, o.b0, o.b1, x.b0; x.b1 is
        # free, so dense m12 (ps_x) starts the instant sweep 1 stops. Each
        # later dense call's pool is chosen so its bank was evicted longest
        # ago. The stats chain + zt4 evicts queue on ACT/DVE underneath.
        z12 = dense(12, ps_x)
        emit_stats_tail()
        zt4 = []
        for m in range(M0):
            zt = ztp.tile([P, E], bf16, name="zt")
            if m % 2 == 0:
                nc.scalar.copy(zt[:], zps4[m][:])
            else:
                nc.vector.tensor_copy(zt[:], zps4[m][:])
            zt4.append(zt)
        z13 = dense(13, ps_z)
        z5 = dense(5, ps_z)
        z6 = dense(6, ps_o)
        emit_wmm(12, z12, act_evict=True)
        emit_wmm_from_zt(0, zt4[0])
        emit_wmm(13, z13, act_evict=False)
        emit_wmm_from_zt(1, zt4[1])
        z7 = dense(7, ps_o)
        emit_wmm(5, z5, act_evict=True)
        emit_wmm_from_zt(2, zt4[2])
        z8 = dense(8, ps_x)
        emit_wmm(6, z6, act_evict=False)
        emit_wmm_from_zt(3, zt4[3])
        z9 = dense(9, ps_x)
        emit_wmm(7, z7, act_evict=True)
        emit_wmm_from_zt(4, zt4[4])
        emit_mm2_group(0, split_evict=True)
        z10 = dense(10, ps_z)
        emit_wmm(8, z8, act_evict=False)
        emit_mm2_group(1, split_evict=True)
        z11 = dense(11, ps_z)
        emit_wmm(9, z9, act_evict=True)
        emit_wmm(10, z10, act_evict=False)
        emit_wmm(11, z11, act_evict=True)
        emit_mm2_group(2, split_evict=True)
        emit_mm2_group(3, split_evict=True)
        emit_mm2_group(4, last=True, split_evict=True)

    nc.finalize()
    return nc


_NC = None


def _get_nc():
    global _NC
    if _NC is None:
        _NC = _build_nc()
    return _NC


def _in_maps(x, G, G1, weight, bias):
    x = np.ascontiguousarray(x, dtype=np.float32)
    G = np.ascontiguousarray(G, dtype=np.float32)
    G1 = np.ascontiguousarray(G1, dtype=np.float32)
    weight = np.ascontiguousarray(weight, dtype=np.float32)
    bias = np.ascontiguousarray(bias, dtype=np.float32)

    # x[t,b,n,f] -> packed [T, P, (k b f)]: partition row p holds the
    # k-tile-major concat of x[t, :, k*128+p, :] (one contiguous HBM run)
    xh = np.ascontiguousarray(
        x.reshape(T, B, NT, P, F).transpose(0, 3, 2, 1, 4)
    ).reshape(T, P, NT * BF).astype(F8)
    # Gc = [G | G1[t]] packed as [T, P, (k e)]; transpose as [T, P, (j n)]
    gc_np = np.concatenate(
        [np.broadcast_to(G[None], (T, N, 256)), G1], axis=2
    )
    gch = np.ascontiguousarray(
        gc_np.reshape(T, NT, P, E).transpose(0, 2, 1, 3)
    ).reshape(T, P, NT * E).astype(F8)
    gcth = np.ascontiguousarray(
        gc_np.transpose(0, 2, 1).reshape(T, ET, P, N).transpose(0, 2, 1, 3)
    ).reshape(T, P, ET * N).astype(F8)
    # blockdiag(W, W)/SG built on host (compensates the 16x in gs)
    bdw_h = np.zeros((P, P), dtype=BF16)
    bdw_h[:F, :F] = (weight / SG).astype(BF16)
    bdw_h[F:, F:] = (weight / SG).astype(BF16)
    b2_h = (np.tile(bias, 2) / SG).reshape(1, P).astype(np.float32)

    maps = []
    for c in range(T):
        maps.append(
            {
                "xs": xh[c],
                "gc": gch[c],
                "gct": gcth[c],
                "bdw": bdw_h,
                "b2": b2_h,
            }
        )
    return maps


def kernel(x, G, G1, weight, bias):
    nc = _get_nc()
    res = bass_utils.run_bass_kernel_spmd(
        nc, _in_maps(x, G, G1, weight, bias), core_ids=list(range(T))
    )
    # os: per core [P, (i b f)] bf16 -> out[b, i*128+p, f] f32
    out = np.stack([np.asarray(r["os"]) for r in res.results], axis=0)
    return np.ascontiguousarray(
        out.reshape(T, P, NT, B, F).transpose(0, 3, 2, 1, 4)
    ).reshape(T * B, N, F).astype(np.float32)
